# revision 16
# baseline (speedup 1.0000x reference)
"""Trainium2 Bass kernel for nn_AM2P_55113020342736 (retrieval_knn).

Math: the reference collapses to a single combined prototype vector v[C]:
  s_fg[b,h,w] = (q[b,:,h,w] . v) / max(||q[b,:,h,w]||, 1e-12)
  logits = stack(-s_fg/T, s_fg/T)
where
  v = BETA/T * Ghat + (1-BETA)/T * sum_m w_m * Phat_m
  Ghat   = G / max(||G||, 1e-12*(fg+EPS)),          G[c] = sum_{s,hw} sf*mask
  Phat_m = Fm / max(||Fm||, 1e-12*(msum_m+EPS)),    Fm[c] = windowed masked sum
(the msum/fg denominators cancel under l2 normalization).

Sharding:
- Support statistics: each core owns one (sample, h-half) slice with ALL 384
  channels, host-transposed to [hw=4608, C]. The prototype matrix
  F[65, 384] = W^T @ feats is computed as 36 TensorE matmuls accumulating in
  one PSUM bank, where W[hw, m] = mask AND window indicator (host-built 0/1
  f32 from the int32 mask/anchor inputs). One 50KB bf16 AllReduce then gives
  every core the complete F; each core redundantly derives the tiny v.
- Query path: data-parallel, 1 query image per core, kept bf16-resident in
  SBUF; per-pixel dot & squared-norm contract over C with M=1 matmuls packed
  4-concurrent into the PE array via 32-aligned tile_position col groups.

All float math runs on device; the host only slices/transposes inputs and
builds integer-derived 0/1 weight matrices and two 65-float coefficient
vectors.
"""

import numpy as np

S, C, H, W = 4, 384, 96, 96
B, M = 8, 64
HW = H * W
NCORES = 8
NP = M + 1                # 64 local prototypes + 1 global
HH = H // 2               # 48 rows per h-half
FH = HH * W               # 4608 support rows per core slice
RADII = (4, 8, 16)
BETA, TEMP, EPS = 0.3, 0.07, 1e-6
NK = 3                    # query c chunks of 128
JW = 512                  # matmul free width
NJ = HW // JW             # 18 j-tiles
QP = 3072                 # query DMA piece
NPIECE = HW // QP         # 3
NSUP = FH // (6 * 128)    # 6 support DMA chunks of [128, 6, C+NP]
CP = C + NP               # 449 combined feats+W row
NPACK = (NJ + 3) // 4     # 5 packs of <=4 j-tiles


def _build_program():
    import concourse.bass as bass
    import concourse.bacc as bacc
    import concourse.mybir as mybir
    import concourse.tile as tile
    from concourse.tile import add_dep_helper

    f32 = mybir.dt.float32
    bf16 = mybir.dt.bfloat16
    add = mybir.AluOpType.add
    mult = mybir.AluOpType.mult
    AF = mybir.ActivationFunctionType

    nc = bacc.Bacc()
    qf = nc.declare_dram_parameter("qf", [C, HW], bf16, isOutput=False)
    sup = nc.declare_dram_parameter("sup", [NSUP, 128, 6, CP], bf16, isOutput=False)
    tiny2 = nc.declare_dram_parameter("tiny2", [NP, 1], f32, isOutput=False)
    wcoef = nc.declare_dram_parameter("wcoef", [NP, 1], f32, isOutput=False)
    out = nc.declare_dram_parameter("out", [2, NJ, JW], f32, isOutput=True)

    groups = [list(range(NCORES))]

    with tile.TileContext(nc) as tc:
        with (
            tc.tile_pool(name="dram", bufs=1, space="DRAM") as dram,
            tc.tile_pool(name="constp", bufs=1) as constp,
            tc.tile_pool(name="qres", bufs=1) as qres,
            tc.tile_pool(name="work", bufs=3) as work,
            tc.tile_pool(name="stage", bufs=3) as stage,
            tc.tile_pool(name="psF", bufs=1, space=bass.MemorySpace.PSUM) as psF,
            tc.tile_pool(name="psA", bufs=3, space=bass.MemorySpace.PSUM) as psA,
        ):
            # ---- constants + activation table preloads ----
            tiny2_sb = constp.tile([NP, 1], f32)
            nc.scalar.dma_start(out=tiny2_sb[:], in_=tiny2[:])
            wcoef_sb = constp.tile([NP, 1], f32)
            nc.scalar.dma_start(out=wcoef_sb[:], in_=wcoef[:])
            ones128 = constp.tile([128, 1], bf16)
            nc.vector.memset(ones128[:], 1.0)
            junk1 = constp.tile([1, 1], f32)
            nc.vector.memset(junk1[:], 1.0)
            warm = constp.tile([1, 1], f32)
            nc.scalar.activation(warm[:], junk1[:], AF.Square)
            nc.scalar.activation(warm[:], junk1[:], AF.Sqrt)
            nc.scalar.activation(warm[:], junk1[:], AF.Copy)
            # PE warm-up: ~5us of junk matmuls so the HAM clock gate opens
            # (1.2 -> 2.4 GHz) before the support matmuls arrive
            wps = psF.tile([1, 512], f32, tag="warmps")
            wsrc = constp.tile([128, 512], bf16)
            nc.vector.memset(wsrc[:], 0.0)
            for _ in range(12):
                nc.tensor.matmul(wps[:], ones128[:], wsrc[:], start=True, stop=True)

            # ---- support phase: F_partial[NP, C] = W^T @ feats ----
            # sup rows are [feats(C) | W(NP)] so one DMA feeds both operands
            fps = psF.tile([NP, C], f32, tag="fps")
            fts = []
            last_sup_dma = None
            for d in range(NSUP):
                ft = work.tile([128, 6, CP], bf16, tag="ft", bufs=NSUP)
                last_sup_dma = nc.sync.dma_start(out=ft[:], in_=sup[d])
                fts.append(ft)
            for d in range(NSUP):
                for j in range(6):
                    nc.tensor.matmul(
                        fps[:], fts[d][:, j, C:], fts[d][:, j, :C],
                        start=(d == 0 and j == 0),
                        stop=(d == NSUP - 1 and j == 5),
                    )
            fpart = constp.tile([NP, C], bf16)
            nc.vector.tensor_copy(fpart[:], fps[:])
            ar_in = dram.tile([NP, C], bf16)
            ar_out = dram.tile([NCORES, NP, C], bf16, addr_space="Shared")
            nc.gpsimd.dma_start(out=ar_in[:], in_=fpart[:])
            nc.gpsimd.collective_compute(
                "AllGather", mybir.AluOpType.bypass, replica_groups=groups,
                ins=[ar_in.opt()], outs=[ar_out.opt()],
            )
            # sum the 8 gathered partials locally: pairwise tree on DVE+GpSimd
            Fg = constp.tile([NP, NCORES, C], bf16)
            ar_flat = bass.AP(ar_out[0].tensor, 0,
                              [[C, NP], [NP * C, NCORES], [1, C]])
            nc.gpsimd.dma_start(out=Fg[:, :, :], in_=ar_flat)
            Fh = constp.tile([NP, 4, C], bf16)
            for u in range(4):
                nc.vector.tensor_tensor(
                    out=Fh[:, u, :], in0=Fg[:, 2 * u, :], in1=Fg[:, 2 * u + 1, :],
                    op=add)
            Fq = constp.tile([NP, 2, C], f32)
            nc.vector.tensor_tensor(
                out=Fq[:, 0, :], in0=Fh[:, 0, :], in1=Fh[:, 1, :], op=add)
            nc.vector.tensor_tensor(
                out=Fq[:, 1, :], in0=Fh[:, 2, :], in1=Fh[:, 3, :], op=add)
            F = constp.tile([NP, C], f32)
            nc.vector.tensor_tensor(
                out=F[:], in0=Fq[:, 0, :], in1=Fq[:, 1, :], op=add)

            # ---- query phase: stream pieces, square on DVE+ACT ----
            # qf transport is held until the support chunks have landed so the
            # support phase (which gates the collective) gets HBM bandwidth.
            qb = [qres.tile([128, HW], bf16, name=f"qb{k}") for k in range(NK)]
            q2 = [qres.tile([128, HW], bf16, name=f"q2{k}") for k in range(NK)]
            VSQ = 2048   # DVE share of each 3072 piece; ACT takes the rest
            for p in range(NPIECE):
                for k in range(NK):
                    qslice = qb[k][:, p * QP : (p + 1) * QP]
                    qdma = nc.sync.dma_start(
                        out=qslice, in_=qf[k * 128 : (k + 1) * 128, p * QP : (p + 1) * QP]
                    )
                    if p == 0 and k == 0:
                        add_dep_helper(qdma.ins, last_sup_dma.ins,
                                       reason="hold qf until sup chunks land")
                    o = p * QP
                    nc.vector.tensor_tensor(
                        out=q2[k][:, o : o + VSQ],
                        in0=qb[k][:, o : o + VSQ],
                        in1=qb[k][:, o : o + VSQ], op=mult)
                    nc.scalar.activation(
                        q2[k][:, o + VSQ : o + QP],
                        qb[k][:, o + VSQ : o + QP], AF.Square)

            # ---- norm2: packed M=1 matmuls, 4 j-tiles per PE pass ----
            n2c = constp.tile([NJ, JW], f32)
            for p in range(NPACK):
                nt = min(4, NJ - 4 * p)
                nps = psA.tile([128, JW], f32, tag="pk", name=f"nps{p}")
                for k in range(NK):
                    for t in range(nt):
                        j = 4 * p + t
                        nc.tensor.matmul(
                            nps[32 * t : 32 * t + 1, :], ones128[:],
                            q2[k][:, j * JW : (j + 1) * JW],
                            start=(k == 0), stop=(k == NK - 1),
                            tile_position=(0, 32 * t),
                        )
                sg = stage.tile([128, JW], f32, tag="sg", name=f"nsg{p}")
                if p % 2 == 0:
                    nc.vector.tensor_copy(sg[:], nps[:])
                else:
                    nc.scalar.copy(sg[:], nps[:])
                nc.sync.dma_start(
                    out=n2c[4 * p : 4 * p + nt, :], in_=sg[0 : 32 * nt : 32, :])

            # rden = 1/max(sqrt(n2), 1e-12); nrden = -rden  (all pre-collective)
            den = constp.tile([NJ, JW], f32)
            nc.scalar.sqrt(den[:], n2c[:])
            nc.vector.tensor_scalar_max(den[:], den[:], 1e-12)
            rden = constp.tile([NJ, JW], f32)
            nc.vector.reciprocal_approx_fast(rden[:], den[:])
            nrden = constp.tile([NJ, JW], f32)
            nc.scalar.mul(nrden[:], rden[:], -1.0)

            # ---- coef_m = wcoef_m / sqrt(||F_m||^2 + tiny2_m);  v = F^T coef ----
            junkF = constp.tile([NP, C], bf16)
            n2p = constp.tile([NP, 1], f32)
            nc.scalar.activation(junkF[:], F[:], AF.Square, accum_out=n2p[:])
            den65 = constp.tile([NP, 1], f32)
            nc.scalar.activation(den65[:], n2p[:], AF.Sqrt, bias=tiny2_sb[:])
            r65 = constp.tile([NP, 1], f32)
            nc.vector.reciprocal(r65[:], den65[:])
            coefb = constp.tile([NP, 1], f32)
            nc.vector.tensor_tensor(out=coefb[:], in0=r65[:], in1=wcoef_sb[:], op=mult)

            vps = psF.tile([128, NK], f32, tag="vps")
            for k in range(NK):
                nc.tensor.matmul(
                    vps[:, k : k + 1], F[:, k * 128 : (k + 1) * 128], coefb[:],
                    start=True, stop=True,
                )
            vb = constp.tile([128, NK], bf16)
            nc.vector.tensor_copy(vb[:], vps[:])

            # ---- dots: packed M=1 matmuls, then compact + epilogue ----
            dc = constp.tile([NJ, JW], f32)
            for p in range(NPACK):
                nt = min(4, NJ - 4 * p)
                dps = psA.tile([128, JW], f32, tag="pk", name=f"dps{p}")
                for k in range(NK):
                    for t in range(nt):
                        j = 4 * p + t
                        nc.tensor.matmul(
                            dps[32 * t : 32 * t + 1, :], vb[:, k : k + 1],
                            qb[k][:, j * JW : (j + 1) * JW],
                            start=(k == 0), stop=(k == NK - 1),
                            tile_position=(0, 32 * t),
                        )
                sg = stage.tile([128, JW], f32, tag="sg", name=f"dsg{p}")
                if p % 2 == 0:
                    nc.scalar.copy(sg[:], dps[:])
                else:
                    nc.vector.tensor_copy(sg[:], dps[:])
                nc.sync.dma_start(
                    out=dc[4 * p : 4 * p + nt, :], in_=sg[0 : 32 * nt : 32, :])

            # ---- epilogue: s1 = dots * rden; s0 = -s1 ----
            s1 = constp.tile([NJ, JW], f32)
            nc.vector.tensor_tensor(out=s1[:], in0=dc[:], in1=rden[:], op=mult)
            s0 = constp.tile([NJ, JW], f32)
            nc.scalar.mul(s0[:], s1[:], -1.0)
            nc.sync.dma_start(out=out[1], in_=s1[:])
            nc.sync.dma_start(out=out[0], in_=s0[:])

    nc.finalize()
    return nc


def prepare(support_feats, support_masks, query_feats, anchor_pos,
            anchor_sample, anchor_radius):
    """Host prep: returns (nc, in_maps)."""
    mask = support_masks[:, 0].astype(np.float32)          # [S,H,W]
    fg = float(np.float32(mask.sum()))

    # integral image of mask for windowed fg counts (host, int bookkeeping)
    ii = np.zeros((S, H + 1, W + 1), np.float64)
    ii[:, 1:, 1:] = mask.astype(np.float64).cumsum(1).cumsum(2)

    windows, msums = [], []
    for m in range(M):
        y, x = int(anchor_pos[m, 0]), int(anchor_pos[m, 1])
        s = int(anchor_sample[m])
        r = RADII[int(anchor_radius[m])]
        y1, y2 = max(y - r, 0), min(y + r, H - 1)
        x1, x2 = max(x - r, 0), min(x + r, W - 1)
        windows.append((s, y1, y2, x1, x2))
        msums.append(ii[s, y2 + 1, x2 + 1] - ii[s, y1, x2 + 1]
                     - ii[s, y2 + 1, x1] + ii[s, y1, x1])
    msums = np.asarray(msums, np.float32)

    # reference's double weight normalization, in f32 like the reference
    lw = msums / (np.float32(msums.sum()) + np.float32(EPS))
    w = lw / (np.float32(lw.sum()) + np.float32(EPS))

    tiny = np.empty((NP, 1), np.float64)
    tiny[:M, 0] = 1e-12 * (msums.astype(np.float64) + EPS)
    tiny[M, 0] = 1e-12 * (fg + EPS)
    wc = np.empty((NP, 1), np.float64)
    wc[:M, 0] = (1.0 - BETA) * w.astype(np.float64) / TEMP
    wc[M, 0] = BETA / TEMP
    tiny2 = (tiny * tiny).astype(np.float32)
    wcoef = wc.astype(np.float32)

    nc = _build_program()

    import ml_dtypes
    qfv = query_feats.reshape(B, C, HW)
    in_maps = []
    for i in range(NCORES):
        s, h = i // 2, i % 2
        # feats slice [C, HH, W] -> transposed [FH, C]
        fsl = support_feats[s, :, h * HH : (h + 1) * HH, :].reshape(C, FH)
        # W[hw, m] = mask AND (hw in window of anchor m with s_m == s);
        # col 64 = mask (global proto)
        msl = mask[s, h * HH : (h + 1) * HH, :]               # [HH, W]
        wm = np.zeros((HH, W, NP), np.float32)
        wm[:, :, M] = msl
        for m, (sm, y1, y2, x1, x2) in enumerate(windows):
            if sm != s:
                continue
            yl = max(y1 - h * HH, 0)
            yh = min(y2 - h * HH, HH - 1)
            if yl > yh:
                continue
            wm[yl : yh + 1, x1 : x2 + 1, m] = msl[yl : yh + 1, x1 : x2 + 1]
        supc = np.concatenate([fsl.T, wm.reshape(FH, NP)], axis=1)  # [FH, CP]
        supb = supc.astype(ml_dtypes.bfloat16)
        in_maps.append({
            "qf": np.ascontiguousarray(qfv[i].astype(ml_dtypes.bfloat16)),
            "sup": np.ascontiguousarray(supb).reshape(NSUP, 128, 6, CP),
            "tiny2": tiny2,
            "wcoef": wcoef,
        })
    return nc, in_maps


def assemble(results):
    outs = [np.asarray(results[i]["out"], np.float32).reshape(2, H, W)
            for i in range(NCORES)]
    return np.stack(outs, axis=0)


def kernel(support_feats, support_masks, query_feats, anchor_pos,
           anchor_sample, anchor_radius):
    from concourse.bass_utils import run_bass_kernel_spmd

    nc, in_maps = prepare(support_feats, support_masks, query_feats,
                          anchor_pos, anchor_sample, anchor_radius)
    res = run_bass_kernel_spmd(nc, in_maps, core_ids=list(range(NCORES)))
    return assemble(res.results)


if __name__ == "__main__":
    pass


# revision 17
# speedup vs baseline: 1.1505x; 1.1505x over previous
"""Trainium2 Bass kernel for nn_AM2P_55113020342736 (retrieval_knn).

Math: the reference collapses to a single combined prototype vector v[C]:
  s_fg[b,h,w] = (q[b,:,h,w] . v) / max(||q[b,:,h,w]||, 1e-12)
  logits = stack(-s_fg/T, s_fg/T)
where
  v = BETA/T * Ghat + (1-BETA)/T * sum_m w_m * Phat_m
  Ghat   = G / max(||G||, 1e-12*(fg+EPS)),          G[c] = sum_{s,hw} sf*mask
  Phat_m = Fm / max(||Fm||, 1e-12*(msum_m+EPS)),    Fm[c] = windowed masked sum
(the msum/fg denominators cancel under l2 normalization).

Sharding:
- Support statistics: each core owns one (sample, h-half) slice with ALL 384
  channels, host-transposed to [hw=4608, C]. The prototype matrix
  F[65, 384] = W^T @ feats is computed as 36 TensorE matmuls accumulating in
  one PSUM bank, where W[hw, m] = mask AND window indicator (host-built 0/1
  f32 from the int32 mask/anchor inputs). One 50KB bf16 AllReduce then gives
  every core the complete F; each core redundantly derives the tiny v.
- Query path: data-parallel, 1 query image per core, kept bf16-resident in
  SBUF; per-pixel dot & squared-norm contract over C with M=1 matmuls packed
  4-concurrent into the PE array via 32-aligned tile_position col groups.

All float math runs on device; the host only slices/transposes inputs and
builds integer-derived 0/1 weight matrices and two 65-float coefficient
vectors.
"""

import numpy as np

S, C, H, W = 4, 384, 96, 96
B, M = 8, 64
HW = H * W
NCORES = 8
NP = M + 1                # 64 local prototypes + 1 global
HH = H // 2               # 48 rows per h-half
FH = HH * W               # 4608 support rows per core slice
RADII = (4, 8, 16)
BETA, TEMP, EPS = 0.3, 0.07, 1e-6
NK = 3                    # query c chunks of 128
JW = 512                  # matmul free width
NJ = HW // JW             # 18 j-tiles
QP = 3072                 # query DMA piece
NPIECE = HW // QP         # 3
NSUP = FH // (6 * 128)    # 6 support DMA chunks of [128, 6, C+NP]
CP = C + NP               # 449 combined feats+W row
NPACK = (NJ + 3) // 4     # 5 packs of <=4 j-tiles


def _build_program():
    import concourse.bass as bass
    import concourse.bacc as bacc
    import concourse.mybir as mybir
    import concourse.tile as tile
    from concourse.tile import add_dep_helper

    f32 = mybir.dt.float32
    bf16 = mybir.dt.bfloat16
    add = mybir.AluOpType.add
    mult = mybir.AluOpType.mult
    AF = mybir.ActivationFunctionType

    nc = bacc.Bacc()
    qf = nc.declare_dram_parameter("qf", [C, HW], bf16, isOutput=False)
    sup = nc.declare_dram_parameter("sup", [NSUP, 128, 6, CP], bf16, isOutput=False)
    tiny2 = nc.declare_dram_parameter("tiny2", [NP, 1], f32, isOutput=False)
    wcoef = nc.declare_dram_parameter("wcoef", [NP, 1], f32, isOutput=False)
    out = nc.declare_dram_parameter("out", [2, NJ, JW], f32, isOutput=True)

    groups = [list(range(NCORES))]

    with tile.TileContext(nc) as tc:
        with (
            tc.tile_pool(name="dram", bufs=1, space="DRAM") as dram,
            tc.tile_pool(name="constp", bufs=1) as constp,
            tc.tile_pool(name="qres", bufs=1) as qres,
            tc.tile_pool(name="work", bufs=3) as work,
            tc.tile_pool(name="stage", bufs=3) as stage,
            tc.tile_pool(name="psF", bufs=1, space=bass.MemorySpace.PSUM) as psF,
            tc.tile_pool(name="psA", bufs=3, space=bass.MemorySpace.PSUM) as psA,
        ):
            # ---- constants + activation table preloads ----
            tiny2_sb = constp.tile([NP, 1], f32)
            nc.scalar.dma_start(out=tiny2_sb[:], in_=tiny2[:])
            wcoef_sb = constp.tile([NP, 1], f32)
            nc.scalar.dma_start(out=wcoef_sb[:], in_=wcoef[:])
            ones128 = constp.tile([128, 1], bf16)
            nc.vector.memset(ones128[:], 1.0)
            junk1 = constp.tile([1, 1], f32)
            nc.vector.memset(junk1[:], 1.0)
            warm = constp.tile([1, 1], f32)
            nc.scalar.activation(warm[:], junk1[:], AF.Square)
            nc.scalar.activation(warm[:], junk1[:], AF.Sqrt)
            nc.scalar.activation(warm[:], junk1[:], AF.Copy)

            # ---- support phase: F_partial[NP, C] = W^T @ feats ----
            # sup rows are [feats(C) | W(NP)] so one DMA feeds both operands
            fps = psF.tile([NP, C], f32, tag="fps")
            fts = []
            last_sup_dma = None
            for d in range(NSUP):
                ft = work.tile([128, 6, CP], bf16, tag="ft", bufs=NSUP)
                last_sup_dma = nc.sync.dma_start(out=ft[:], in_=sup[d])
                fts.append(ft)
            for d in range(NSUP):
                for j in range(6):
                    nc.tensor.matmul(
                        fps[:], fts[d][:, j, C:], fts[d][:, j, :C],
                        start=(d == 0 and j == 0),
                        stop=(d == NSUP - 1 and j == 5),
                    )
            fpart = constp.tile([NP, C], bf16)
            nc.vector.tensor_copy(fpart[:], fps[:])
            ar_in = dram.tile([NP, C], bf16)
            ar_out = dram.tile([NCORES, NP, C], bf16, addr_space="Shared")
            nc.gpsimd.dma_start(out=ar_in[:], in_=fpart[:])
            nc.gpsimd.collective_compute(
                "AllGather", mybir.AluOpType.bypass, replica_groups=groups,
                ins=[ar_in.opt()], outs=[ar_out.opt()],
            )
            # sum the 8 gathered partials locally: pairwise tree on DVE+GpSimd
            Fg = constp.tile([NP, NCORES, C], bf16)
            ar_flat = bass.AP(ar_out[0].tensor, 0,
                              [[C, NP], [NP * C, NCORES], [1, C]])
            nc.gpsimd.dma_start(out=Fg[:, :, :], in_=ar_flat)
            Fh = constp.tile([NP, 4, C], bf16)
            for u in range(4):
                nc.vector.tensor_tensor(
                    out=Fh[:, u, :], in0=Fg[:, 2 * u, :], in1=Fg[:, 2 * u + 1, :],
                    op=add)
            Fq = constp.tile([NP, 2, C], f32)
            nc.vector.tensor_tensor(
                out=Fq[:, 0, :], in0=Fh[:, 0, :], in1=Fh[:, 1, :], op=add)
            nc.vector.tensor_tensor(
                out=Fq[:, 1, :], in0=Fh[:, 2, :], in1=Fh[:, 3, :], op=add)
            F = constp.tile([NP, C], f32)
            nc.vector.tensor_tensor(
                out=F[:], in0=Fq[:, 0, :], in1=Fq[:, 1, :], op=add)

            # ---- query phase: stream pieces, square on DVE+ACT ----
            # qf transport is held until the support chunks have landed so the
            # support phase (which gates the collective) gets HBM bandwidth.
            qb = [qres.tile([128, HW], bf16, name=f"qb{k}") for k in range(NK)]
            q2 = [qres.tile([128, HW], bf16, name=f"q2{k}") for k in range(NK)]
            VSQ = 2048   # DVE share of each 3072 piece; ACT takes the rest
            for p in range(NPIECE):
                for k in range(NK):
                    qslice = qb[k][:, p * QP : (p + 1) * QP]
                    qdma = nc.sync.dma_start(
                        out=qslice, in_=qf[k * 128 : (k + 1) * 128, p * QP : (p + 1) * QP]
                    )
                    if p == 0 and k == 0:
                        add_dep_helper(qdma.ins, last_sup_dma.ins,
                                       reason="hold qf until sup chunks land")
                    o = p * QP
                    nc.vector.tensor_tensor(
                        out=q2[k][:, o : o + VSQ],
                        in0=qb[k][:, o : o + VSQ],
                        in1=qb[k][:, o : o + VSQ], op=mult)
                    nc.scalar.activation(
                        q2[k][:, o + VSQ : o + QP],
                        qb[k][:, o + VSQ : o + QP], AF.Square)

            # ---- norm2: packed M=1 matmuls, 4 j-tiles per PE pass ----
            n2c = constp.tile([NJ, JW], f32)
            for p in range(NPACK):
                nt = min(4, NJ - 4 * p)
                nps = psA.tile([128, JW], f32, tag="pk", name=f"nps{p}")
                for k in range(NK):
                    for t in range(nt):
                        j = 4 * p + t
                        nc.tensor.matmul(
                            nps[32 * t : 32 * t + 1, :], ones128[:],
                            q2[k][:, j * JW : (j + 1) * JW],
                            start=(k == 0), stop=(k == NK - 1),
                            tile_position=(0, 32 * t),
                        )
                sg = stage.tile([128, JW], f32, tag="sg", name=f"nsg{p}")
                if p % 2 == 0:
                    nc.vector.tensor_copy(sg[:], nps[:])
                else:
                    nc.scalar.copy(sg[:], nps[:])
                nc.sync.dma_start(
                    out=n2c[4 * p : 4 * p + nt, :], in_=sg[0 : 32 * nt : 32, :])

            # rden = 1/max(sqrt(n2), 1e-12); nrden = -rden  (all pre-collective)
            den = constp.tile([NJ, JW], f32)
            nc.scalar.sqrt(den[:], n2c[:])
            nc.vector.tensor_scalar_max(den[:], den[:], 1e-12)
            rden = constp.tile([NJ, JW], f32)
            nc.vector.reciprocal_approx_fast(rden[:], den[:])
            nrden = constp.tile([NJ, JW], f32)
            nc.scalar.mul(nrden[:], rden[:], -1.0)

            # ---- coef_m = wcoef_m / sqrt(||F_m||^2 + tiny2_m);  v = F^T coef ----
            junkF = constp.tile([NP, C], bf16)
            n2p = constp.tile([NP, 1], f32)
            nc.scalar.activation(junkF[:], F[:], AF.Square, accum_out=n2p[:])
            den65 = constp.tile([NP, 1], f32)
            nc.scalar.activation(den65[:], n2p[:], AF.Sqrt, bias=tiny2_sb[:])
            r65 = constp.tile([NP, 1], f32)
            nc.vector.reciprocal(r65[:], den65[:])
            coefb = constp.tile([NP, 1], f32)
            nc.vector.tensor_tensor(out=coefb[:], in0=r65[:], in1=wcoef_sb[:], op=mult)

            vps = psF.tile([128, NK], f32, tag="vps")
            for k in range(NK):
                nc.tensor.matmul(
                    vps[:, k : k + 1], F[:, k * 128 : (k + 1) * 128], coefb[:],
                    start=True, stop=True,
                )
            vb = constp.tile([128, NK], bf16)
            nc.vector.tensor_copy(vb[:], vps[:])

            # ---- dots: packed M=1 matmuls, then compact + epilogue ----
            dc = constp.tile([NJ, JW], f32)
            for p in range(NPACK):
                nt = min(4, NJ - 4 * p)
                dps = psA.tile([128, JW], f32, tag="pk", name=f"dps{p}")
                for k in range(NK):
                    for t in range(nt):
                        j = 4 * p + t
                        nc.tensor.matmul(
                            dps[32 * t : 32 * t + 1, :], vb[:, k : k + 1],
                            qb[k][:, j * JW : (j + 1) * JW],
                            start=(k == 0), stop=(k == NK - 1),
                            tile_position=(0, 32 * t),
                        )
                sg = stage.tile([128, JW], f32, tag="sg", name=f"dsg{p}")
                if p % 2 == 0:
                    nc.scalar.copy(sg[:], dps[:])
                else:
                    nc.vector.tensor_copy(sg[:], dps[:])
                nc.sync.dma_start(
                    out=dc[4 * p : 4 * p + nt, :], in_=sg[0 : 32 * nt : 32, :])

            # ---- epilogue: s1 = dots * rden; s0 = -s1 ----
            s1 = constp.tile([NJ, JW], f32)
            nc.vector.tensor_tensor(out=s1[:], in0=dc[:], in1=rden[:], op=mult)
            s0 = constp.tile([NJ, JW], f32)
            nc.scalar.mul(s0[:], s1[:], -1.0)
            nc.sync.dma_start(out=out[1], in_=s1[:])
            nc.sync.dma_start(out=out[0], in_=s0[:])

    nc.finalize()
    return nc


def prepare(support_feats, support_masks, query_feats, anchor_pos,
            anchor_sample, anchor_radius):
    """Host prep: returns (nc, in_maps)."""
    mask = support_masks[:, 0].astype(np.float32)          # [S,H,W]
    fg = float(np.float32(mask.sum()))

    # integral image of mask for windowed fg counts (host, int bookkeeping)
    ii = np.zeros((S, H + 1, W + 1), np.float64)
    ii[:, 1:, 1:] = mask.astype(np.float64).cumsum(1).cumsum(2)

    windows, msums = [], []
    for m in range(M):
        y, x = int(anchor_pos[m, 0]), int(anchor_pos[m, 1])
        s = int(anchor_sample[m])
        r = RADII[int(anchor_radius[m])]
        y1, y2 = max(y - r, 0), min(y + r, H - 1)
        x1, x2 = max(x - r, 0), min(x + r, W - 1)
        windows.append((s, y1, y2, x1, x2))
        msums.append(ii[s, y2 + 1, x2 + 1] - ii[s, y1, x2 + 1]
                     - ii[s, y2 + 1, x1] + ii[s, y1, x1])
    msums = np.asarray(msums, np.float32)

    # reference's double weight normalization, in f32 like the reference
    lw = msums / (np.float32(msums.sum()) + np.float32(EPS))
    w = lw / (np.float32(lw.sum()) + np.float32(EPS))

    tiny = np.empty((NP, 1), np.float64)
    tiny[:M, 0] = 1e-12 * (msums.astype(np.float64) + EPS)
    tiny[M, 0] = 1e-12 * (fg + EPS)
    wc = np.empty((NP, 1), np.float64)
    wc[:M, 0] = (1.0 - BETA) * w.astype(np.float64) / TEMP
    wc[M, 0] = BETA / TEMP
    tiny2 = (tiny * tiny).astype(np.float32)
    wcoef = wc.astype(np.float32)

    nc = _build_program()

    import ml_dtypes
    qfv = query_feats.reshape(B, C, HW)
    in_maps = []
    for i in range(NCORES):
        s, h = i // 2, i % 2
        # feats slice [C, HH, W] -> transposed [FH, C]
        fsl = support_feats[s, :, h * HH : (h + 1) * HH, :].reshape(C, FH)
        # W[hw, m] = mask AND (hw in window of anchor m with s_m == s);
        # col 64 = mask (global proto)
        msl = mask[s, h * HH : (h + 1) * HH, :]               # [HH, W]
        wm = np.zeros((HH, W, NP), np.float32)
        wm[:, :, M] = msl
        for m, (sm, y1, y2, x1, x2) in enumerate(windows):
            if sm != s:
                continue
            yl = max(y1 - h * HH, 0)
            yh = min(y2 - h * HH, HH - 1)
            if yl > yh:
                continue
            wm[yl : yh + 1, x1 : x2 + 1, m] = msl[yl : yh + 1, x1 : x2 + 1]
        supc = np.concatenate([fsl.T, wm.reshape(FH, NP)], axis=1)  # [FH, CP]
        supb = supc.astype(ml_dtypes.bfloat16)
        in_maps.append({
            "qf": np.ascontiguousarray(qfv[i].astype(ml_dtypes.bfloat16)),
            "sup": np.ascontiguousarray(supb).reshape(NSUP, 128, 6, CP),
            "tiny2": tiny2,
            "wcoef": wcoef,
        })
    return nc, in_maps


def assemble(results):
    outs = [np.asarray(results[i]["out"], np.float32).reshape(2, H, W)
            for i in range(NCORES)]
    return np.stack(outs, axis=0)


def kernel(support_feats, support_masks, query_feats, anchor_pos,
           anchor_sample, anchor_radius):
    from concourse.bass_utils import run_bass_kernel_spmd

    nc, in_maps = prepare(support_feats, support_masks, query_feats,
                          anchor_pos, anchor_sample, anchor_radius)
    res = run_bass_kernel_spmd(nc, in_maps, core_ids=list(range(NCORES)))
    return assemble(res.results)


if __name__ == "__main__":
    pass
